# revision 1
# baseline (speedup 1.0000x reference)
"""CapsuleLayer dynamic-routing kernel for 8 Trainium2 NeuronCores.

Sharding: data-parallel over batch (16 batches/core), weight replicated.
  u_hat[b,c,n,s] = sum_i W[c,n,s,i] * x[b,i,c]   (PE, fp32r, block-diag x)
  3 routing iterations; the b_ij update takes a mean over the full batch
  via one AllReduce per iteration (skipped on the last).

On-chip: u_hat kept in SBUF as A[c%128, chunk, b, n, s] (9 chunks of 128
channels).  s_j is a PE pass (c_ij stationary, u_hat moving); the
agreement <u_hat, v> is a GPSIMD multiply + DVE segmented reduce.
"""

import sys

sys.path.insert(0, "/opt/trn_rl_repo")

import numpy as np

B, IN_UNIT, IN_CHANNEL = 128, 16, 1152
NUM_UNIT, UNIT_SIZE = 16, 16
NCORES = 8
BL = B // NCORES               # 16 batches per core
NGROUP = IN_CHANNEL // 8       # 144 groups of 8 channels
NCHUNK = IN_CHANNEL // 128     # 9 c-chunks
NS = NUM_UNIT * UNIT_SIZE      # 256
FREE = BL * NS                 # 4096 = (b, n, s) free size per chunk

_cache = {}


def _build(single_core=False, niters=3, skip_prod=False):
    import concourse.bass as bass
    import concourse.bacc as bacc
    import concourse.mybir as mybir
    import concourse.tile as tile

    f32 = mybir.dt.float32
    f32r = mybir.dt.float32r
    ALU = mybir.AluOpType
    AX = mybir.AxisListType

    def sub(ap, off, dims, cast=None):
        a = bass.AP(ap.tensor, ap.offset + off, [list(d) for d in dims])
        return a.bitcast(cast) if cast is not None else a

    nc = bacc.Bacc("TRN2", target_bir_lowering=False, debug=False,
                   num_devices=1 if single_core else NCORES)

    wr_t = nc.dram_tensor("wr", [NGROUP * 128, 256], f32, kind="ExternalInput")
    xc_t = nc.dram_tensor("xc", [IN_CHANNEL, IN_UNIT, BL], f32, kind="ExternalInput")
    cij1_t = nc.dram_tensor("cij1", [128, NUM_UNIT], f32, kind="ExternalInput")
    vout_t = nc.dram_tensor("vout", [NUM_UNIT, BL * UNIT_SIZE], f32,
                            kind="ExternalOutput")

    with tile.TileContext(nc) as tc:
        with tc.tile_pool(name="apool", bufs=1) as apool, \
             tc.tile_pool(name="persist", bufs=1) as persist, \
             tc.tile_pool(name="drampool", bufs=1, space="DRAM") as drampool:
            # u_hat, fp32r, [c_part, chunk, b, n, s]
            A = apool.tile([128, NCHUNK, BL, NUM_UNIT, UNIT_SIZE], f32r)
            Aap = A[:]
            pstA = Aap.ap[0][0]
            b_ij = persist.tile([128, NCHUNK, NUM_UNIT], f32)
            cij_u = persist.tile([128, NUM_UNIT], f32r)   # uniform 1/16
            nc.gpsimd.memset(b_ij[:], 0.0)
            nc.sync.dma_start(cij_u[:], cij1_t[:].bitcast(f32r))

            # ---------------- production ----------------
            uhd = drampool.tile([NGROUP * 128, 256], f32)    # u_hat bounce
            with tc.tile_pool(name="bdp", bufs=1) as bdp, \
                 tc.tile_pool(name="wp", bufs=1) as wp, \
                 tc.tile_pool(name="stgp", bufs=1) as stgp, \
                 tc.tile_pool(name="psp", bufs=8, space="PSUM") as psp:
                for sg in (range(NCHUNK) if not skip_prod else []):
                    bd16 = bdp.tile([128, 16, 128], f32r, tag="bd16",
                                    name=f"bd16_{sg}")
                    pstB = bd16[:].ap[0][0]
                    if sg < 1:      # single slot; zero padding persists
                        nc.gpsimd.memset(bd16[:].bitcast(f32), 0.0)
                    for cc in range(8):
                        # block-diag xT: bd16[(cc,i), g, cc*16+b]
                        src = sub(xc_t[:], (8 * 16 * sg + cc) * IN_UNIT * BL,
                                  [[BL, IN_UNIT],
                                   [8 * IN_UNIT * BL, 16],
                                   [1, BL]], cast=f32r)
                        dst = sub(bd16[:], cc * 16 * pstB + cc * 16,
                                  [[pstB, IN_UNIT], [128, 16], [1, BL]])
                        nc.sync.dma_start(dst, src)
                    wts = []
                    for gq in range(4):
                        wt4 = wp.tile([128, 4, 256], f32r, tag=f"w{gq}",
                                      name=f"w{gq}_{sg}")
                        nc.sync.dma_start(
                            wt4[:], sub(wr_t[:], (sg * 16 + gq * 4) * 128 * 256,
                                        [[256, 128], [128 * 256, 4], [1, 256]],
                                        cast=f32r))
                        wts.append(wt4)
                    for half in range(2):
                        stgb = stgp.tile([128, 8, 256], f32, tag="stgb",
                                         bufs=2, name=f"stgb_{sg}_{half}")
                        for g8 in range(8):
                            gg = half * 8 + g8
                            ps = psp.tile([128, 256], f32, tag="ps",
                                          name=f"ps_{sg}_{gg}")
                            nc.tensor.matmul(ps[:], bd16[:, gg, :],
                                             wts[gg // 4][:, gg % 4, :],
                                             start=True, stop=True)
                            if gg % 2 == 0:
                                nc.vector.tensor_copy(stgb[:, g8, :], ps[:])
                            else:
                                nc.scalar.copy(stgb[:, g8, :], ps[:])
                        # (cc,b),(g,n,s) -> DRAM uhd[(g,cc,b), (n,s)]
                        dstu = sub(uhd[:], sg * 16 * 128 * 256
                                   + half * 8 * 8 * 16 * 256,
                                   [[16 * 256, 8], [256, 16],
                                    [8 * 16 * 256, 8], [1, 256]])
                        nc.sync.dma_start(dstu, stgb[:])
                    # readback c-partitioned: A[p, sg, (b,n,s)]
                    dstA = sub(Aap, sg * FREE, [[pstA, 128], [1, FREE]],
                               cast=f32)
                    srcu = sub(uhd[:], sg * 16 * 128 * 256,
                               [[FREE, 128], [1, FREE]])
                    nc.sync.dma_start(dstA, srcu)

            # ---------------- routing ----------------
            with tc.tile_pool(name="rt", bufs=1) as rt, \
                 tc.tile_pool(name="tb", bufs=2) as tb, \
                 tc.tile_pool(name="pss", bufs=1, space="PSUM") as pss:
                vb = rt.tile([128, FREE], f32)
                cij = rt.tile([128, NCHUNK, NUM_UNIT], f32)
                cij_r = rt.tile([128, NCHUNK, NUM_UNIT], f32r)
                smax = rt.tile([128, NCHUNK], f32)
                ssum = rt.tile([128, NCHUNK], f32)
                uv = rt.tile([128, NCHUNK, NUM_UNIT], f32)
                ar_sb = rt.tile([128, NCHUNK, NUM_UNIT], f32)

                for it in range(niters):
                    if it > 0:
                        # softmax over n of b_ij -> cij (f32r via DMA recast)
                        nc.vector.tensor_reduce(smax[:], b_ij[:], axis=AX.X,
                                                op=ALU.max)
                        mb = sub(smax[:], 0,
                                 [[NCHUNK, 128], [1, NCHUNK], [0, NUM_UNIT]])
                        nc.vector.tensor_tensor(cij[:], b_ij[:], mb,
                                                op=ALU.subtract)
                        nc.scalar.activation(cij[:], cij[:],
                                             mybir.ActivationFunctionType.Exp)
                        nc.vector.tensor_reduce(ssum[:], cij[:], axis=AX.X,
                                                op=ALU.add)
                        nc.vector.reciprocal(ssum[:], ssum[:])
                        sb = sub(ssum[:], 0,
                                 [[NCHUNK, 128], [1, NCHUNK], [0, NUM_UNIT]])
                        nc.vector.tensor_tensor(cij[:], cij[:], sb, op=ALU.mult)
                        nc.sync.dma_start(cij_r[:], cij[:].bitcast(f32r))

                    # s_j: PE pass, c_ij stationary, u_hat moving
                    psj = pss.tile([NUM_UNIT, FREE], f32, tag="psj",
                                   name=f"psj_{it}")
                    pstP = psj[:].ap[0][0]
                    for k in range(NCHUNK):
                        lhs = cij_u[:] if it == 0 else cij_r[:, k, :]
                        for j in range(FREE // 512):
                            nc.tensor.matmul(
                                psj[:, j * 512:(j + 1) * 512], lhs,
                                sub(Aap, k * FREE + j * 512,
                                    [[pstA, 128], [1, 512]]),
                                start=(k == 0), stop=(k == NCHUNK - 1))

                    # diagonal extract: s[n,(b,s)] = psj[n, (b,n,s)]
                    sjf = tb.tile([NUM_UNIT, FREE], f32, tag="big1", bufs=1,
                                  name=f"sjf_{it}")
                    nc.vector.tensor_copy(sjf[:, :FREE // 2],
                                          psj[:, :FREE // 2])
                    nc.scalar.copy(sjf[:, FREE // 2:], psj[:, FREE // 2:])
                    pstS = sjf[:].ap[0][0]
                    s_t = tb.tile([NUM_UNIT, BL, UNIT_SIZE], f32, tag="s_t",
                                  name=f"s_t{it}")
                    pstST = s_t[:].ap[0][0]
                    # one DMA: partition-dim step carries the diagonal offset
                    src = sub(sjf[:], 0,
                              [[pstS + UNIT_SIZE, NUM_UNIT],
                               [NS, BL], [1, UNIT_SIZE]])
                    nc.sync.dma_start(s_t[:], src)

                    # squash over s
                    s2 = tb.tile([NUM_UNIT, BL, UNIT_SIZE], f32, tag="big1",
                                 bufs=1, name=f"s2_{it}")
                    nc.vector.tensor_tensor(s2[:], s_t[:], s_t[:], op=ALU.mult)
                    sq = tb.tile([NUM_UNIT, BL], f32, tag="sq", name=f"sq_{it}")
                    nc.vector.tensor_reduce(sq[:], s2[:], axis=AX.X, op=ALU.add)
                    rsq = tb.tile([NUM_UNIT, BL], f32, tag="rsq",
                                  name=f"rsq_{it}")
                    nc.scalar.sqrt(rsq[:], sq[:])
                    den = tb.tile([NUM_UNIT, BL], f32, tag="den",
                                  name=f"den_{it}")
                    nc.vector.scalar_tensor_tensor(den[:], sq[:], 1.0, rsq[:],
                                                   op0=ALU.add, op1=ALU.mult)
                    nc.vector.reciprocal(den[:], den[:])
                    fac = tb.tile([NUM_UNIT, BL], f32, tag="fac",
                                  name=f"fac_{it}")
                    nc.vector.tensor_tensor(fac[:], sq[:], den[:], op=ALU.mult)
                    v_t = tb.tile([NUM_UNIT, BL, UNIT_SIZE], f32, tag="v_t",
                                  name=f"v_t{it}")
                    pstF = fac[:].ap[0][0]
                    fb = sub(fac[:], 0, [[pstF, NUM_UNIT], [1, BL],
                                         [0, UNIT_SIZE]])
                    nc.vector.tensor_tensor(v_t[:], s_t[:], fb, op=ALU.mult)

                    if it == niters - 1:
                        nc.sync.dma_start(vout_t[:],
                                          sub(v_t[:], 0,
                                              [[v_t[:].ap[0][0], NUM_UNIT],
                                               [1, BL * UNIT_SIZE]]))
                        break

                    # flatten v[n,(b,s)] -> vb[0, (b,n,s)], one DMA per n
                    pstV = v_t[:].ap[0][0]
                    pstVB = vb[:].ap[0][0]
                    for n in range(NUM_UNIT):
                        dstv = sub(vb[:], n * UNIT_SIZE,
                                   [[pstVB, 1], [NS, BL], [1, UNIT_SIZE]])
                        srcv = sub(v_t[:], n * pstV,
                                   [[pstV, 1], [UNIT_SIZE, BL],
                                    [1, UNIT_SIZE]])
                        nc.sync.dma_start(dstv, srcv)
                    nc.gpsimd.partition_broadcast(vb[:, :], vb[0:1, :])

                    # agreement: uv[c,n] = sum_{b,s} u_hat * v
                    QF = FREE // 4          # 1024 = 4 batches
                    for k in range(NCHUNK):
                        rsb = tb.tile([128, 4, 4, NUM_UNIT], f32, tag="rsb",
                                      name=f"rsb_{it}_{k}")
                        for h in range(4):
                            tmp = tb.tile([128, QF], f32, tag="uvt",
                                          name=f"uvt_{it}_{k}_{h}")
                            eng = (nc.gpsimd if (k * 4 + h) % 6 < 5
                                   else nc.vector)
                            eng.tensor_tensor(
                                tmp[:],
                                sub(Aap, k * FREE + h * QF,
                                    [[pstA, 128], [1, QF]], cast=f32),
                                vb[:, h * QF:(h + 1) * QF],
                                op=ALU.mult)
                            pstT = tmp[:].ap[0][0]
                            nc.vector.tensor_reduce(
                                rsb[:, h],
                                sub(tmp[:], 0,
                                    [[pstT, 128], [NS, 4],
                                     [UNIT_SIZE, NUM_UNIT], [1, UNIT_SIZE]]),
                                axis=AX.X, op=ALU.add)
                        pstR = rsb[:].ap[0][0]
                        nc.vector.tensor_reduce(
                            uv[:, k], sub(rsb[:], 0,
                                          [[pstR, 128], [1, NUM_UNIT],
                                           [NUM_UNIT, 16]]),
                            axis=AX.X, op=ALU.add)

                    arbounce_i = drampool.tile([128, NCHUNK * NUM_UNIT], f32,
                                               name=f"arbi_{it}", tag=f"arbi{it}")
                    arbounce_o = drampool.tile([128, NCHUNK * NUM_UNIT], f32,
                                               addr_space="Shared",
                                               name=f"arbo_{it}", tag=f"arbo{it}")
                    nc.gpsimd.dma_start(arbounce_i[:], uv[:])
                    if single_core:
                        nc.gpsimd.dma_start(arbounce_o[:], arbounce_i[:])
                    else:
                        nc.gpsimd.collective_compute(
                            "AllReduce", ALU.add,
                            replica_groups=[list(range(NCORES))],
                            ins=[arbounce_i.opt()], outs=[arbounce_o.opt()])
                    nc.sync.dma_start(ar_sb[:], arbounce_o[:])
                    # b_ij += AR/B
                    nc.vector.scalar_tensor_tensor(b_ij[:], ar_sb[:], 1.0 / B,
                                                   b_ij[:], op0=ALU.mult,
                                                   op1=ALU.add)

    nc.compile()
    return nc


def _prep(x, weight):
    wr = np.ascontiguousarray(
        weight.reshape(NGROUP, 8, NUM_UNIT, UNIT_SIZE, IN_UNIT)
        .transpose(0, 1, 4, 2, 3).reshape(NGROUP * 128, 256)).astype(np.float32)
    cij1 = np.full((128, NUM_UNIT), 1.0 / NUM_UNIT, np.float32)
    in_maps = []
    for c in range(NCORES):
        xs = x[c * BL:(c + 1) * BL]          # [BL, i, C]
        xc = np.ascontiguousarray(xs.transpose(2, 1, 0)).astype(np.float32)
        in_maps.append({"wr": wr, "xc": xc, "cij1": cij1})
    return in_maps


def kernel(x, x_original, weight, mode, epoch, _trace=False):
    from concourse.bass_utils import run_bass_kernel_spmd

    x = np.asarray(x, dtype=np.float32)
    weight = np.asarray(weight, dtype=np.float32)
    if "nc" not in _cache:
        _cache["nc"] = _build()
    nc = _cache["nc"]
    in_maps = _prep(x, weight)
    res = run_bass_kernel_spmd(nc, in_maps, core_ids=list(range(NCORES)),
                               trace=_trace)
    _cache["last_result"] = res
    out = np.empty((B, NUM_UNIT, UNIT_SIZE), np.float32)
    for c in range(NCORES):
        vo = res.results[c]["vout"].reshape(NUM_UNIT, BL, UNIT_SIZE)
        out[c * BL:(c + 1) * BL] = vo.transpose(1, 0, 2)
    return out[..., None]



# revision 11
# speedup vs baseline: 1.6956x; 1.6956x over previous
"""CapsuleLayer dynamic-routing kernel for 8 Trainium2 NeuronCores.

Sharding: data-parallel over batch (16 batches/core), weight replicated.
  u_hat[b,c,n,s] = sum_i W[c,n,s,i] * x[b,i,c]   (PE, bf16, block-diag x)
  3 routing iterations; b_ij update takes a mean over the full batch via
  AllReduce (split in channel-halves so it overlaps compute).

v2 design vs baseline:
  - everything bf16 (tol 2e-2; measured rel err ~1e-3): halves DMA + 2x DVE
  - u_hat transposed to [c-part, (b,n,s)] via one SBUF->SBUF DMA per chunk
    (512B runs) instead of a DRAM round-trip
  - iteration-0 s_j computed during production from the staging tiles with a
    delta/16 stationary (PE idle time), output directly [b,(n,s)] (no diag)
  - agreement = DVE tensor_tensor mult + pairwise fold tree (bf16 2x mode);
    3 chunk-mults offloaded to GPSIMD
  - diag extract for s_j via a single strided DVE copy from PSUM
  - v broadcast via DRAM bounce + stride-0 broadcast DMA
  - tiny keep-warm matmuls gated on DVE steps hold the PE clock at max
"""

import sys

sys.path.insert(0, "/opt/trn_rl_repo")

import numpy as np

B, IN_UNIT, IN_CHANNEL = 128, 16, 1152
NUM_UNIT, UNIT_SIZE = 16, 16
NCORES = 8
BL = B // NCORES               # 16 batches per core
NCHUNK = IN_CHANNEL // 128     # 9 c-chunks
NGRP = 16                      # groups of 8 channels per chunk
NS = NUM_UNIT * UNIT_SIZE      # 256
FREE = BL * NS                 # 4096 = (b, n, s) free size per chunk
POOL_CHUNKS = (6, 7, 8)        # agreement mults done on gpsimd

_cache = {}


def _build(single_core=False, niters=3):
    import concourse.bass as bass
    import concourse.bacc as bacc
    import concourse.mybir as mybir
    import concourse.tile as tile

    f32 = mybir.dt.float32
    bf16 = mybir.dt.bfloat16
    ALU = mybir.AluOpType
    AX = mybir.AxisListType
    ACT = mybir.ActivationFunctionType

    def sub(ap, off, dims, cast=None):
        a = bass.AP(ap.tensor, ap.offset + off, [list(d) for d in dims])
        return a.bitcast(cast) if cast is not None else a

    nc = bacc.Bacc("TRN2", target_bir_lowering=False, debug=False,
                   num_devices=1 if single_core else NCORES)

    wr_t = nc.dram_tensor("wr", [NCHUNK * NGRP * 128, 256], bf16,
                          kind="ExternalInput")
    xbd_t = nc.dram_tensor("xbd", [NCHUNK, 128, NGRP * 128], bf16,
                           kind="ExternalInput")
    sd_t = nc.dram_tensor("sd", [128, BL], bf16, kind="ExternalInput")
    vout_t = nc.dram_tensor("vout", [NUM_UNIT, BL * UNIT_SIZE], f32,
                            kind="ExternalOutput")

    with tile.TileContext(nc) as tc:
        with tc.tile_pool(name="persist", bufs=1) as persist, \
             tc.tile_pool(name="drampool", bufs=1, space="DRAM") as drampool:
            A = persist.tile([128, NCHUNK, BL, NUM_UNIT, UNIT_SIZE], bf16)
            Aap = A[:]
            pstA = Aap.ap[0][0]
            b_ij = persist.tile([128, NCHUNK, NUM_UNIT], f32)
            uv = persist.tile([128, NCHUNK, NUM_UNIT], f32)
            vb = persist.tile([128, BL, NUM_UNIT, UNIT_SIZE], bf16)
            pstVB = vb[:].ap[0][0]
            sd = persist.tile([128, BL], bf16)
            s0 = persist.tile([BL, NS], f32)     # iter-0 s_j, SBUF copy
            nc.gpsimd.memset(b_ij[:], 0.0)
            nc.sync.dma_start(sd[:], sd_t[:])

            # ---------------- production + iter-0 s_j ----------------
            CHW = NGRP * 128 * 256          # uhd elements per chunk
            uhd = drampool.tile([NCHUNK, NGRP, 8, BL, 256], bf16,
                                name="uhd")  # rows (gg, cc, b)
            with tc.tile_pool(name="bdp", bufs=2) as bdp, \
                 tc.tile_pool(name="wp", bufs=2) as wp, \
                 tc.tile_pool(name="stgp", bufs=2) as stgp, \
                 tc.tile_pool(name="psp", bufs=3, space="PSUM") as psp, \
                 tc.tile_pool(name="ps0", bufs=1, space="PSUM") as ps0:
                psj0 = ps0.tile([BL, NS], f32)   # s_j iter0: [b, (n,s)]
                for sg in range(NCHUNK):
                    # dense host-built block-diag xT: bd[(cc,i), (gg, cc*16+b)]
                    bd = bdp.tile([128, NGRP, 128], bf16, tag="bd",
                                  name=f"bd_{sg}")
                    pstB = bd[:].ap[0][0]
                    nc.sync.dma_start(
                        sub(bd[:], 0, [[pstB, 128], [1, NGRP * 128]]),
                        sub(xbd_t[:], sg * 128 * NGRP * 128,
                            [[NGRP * 128, 128], [1, NGRP * 128]]))
                    # weights for the chunk: wt[(cc,i), gq, (n,s)]
                    wt = wp.tile([128, NGRP, 256], bf16, tag="wt",
                                 name=f"wt_{sg}")
                    nc.sync.dma_start(
                        wt[:], sub(wr_t[:], sg * NGRP * 128 * 256,
                                   [[256, 128], [128 * 256, NGRP], [1, 256]]))
                    stg = stgp.tile([128, NGRP, 256], bf16, tag="stg",
                                    name=f"stg_{sg}")
                    pstS = stg[:].ap[0][0]
                    for q in range(4):      # 4 psum tiles of 4 groups
                        ps = psp.tile([128, 4 * 256], f32, tag="pp",
                                      name=f"pp_{sg}_{q}")
                        for g4 in range(4):
                            gg = q * 4 + g4
                            nc.tensor.matmul(ps[:, g4 * 256:(g4 + 1) * 256],
                                             bd[:, gg, :], wt[:, gg, :],
                                             start=True, stop=True)
                        eng = nc.vector if q % 2 == 0 else nc.scalar
                        if q % 2 == 0:
                            eng.tensor_copy(
                                sub(stg[:], q * 4 * 256,
                                    [[pstS, 128], [1, 1024]]),
                                ps[:])
                        else:
                            eng.copy(
                                sub(stg[:], q * 4 * 256,
                                    [[pstS, 128], [1, 1024]]),
                                ps[:])
                    # iter-0 s_j partials: psj0[b,(n,s)] += sd^T @ stg
                    for gg in range(NGRP):
                        nc.tensor.matmul(psj0[:], sd[:], stg[:, gg, :],
                                         start=(sg == 0 and gg == 0),
                                         stop=(sg == NCHUNK - 1
                                               and gg == NGRP - 1))
                    # bounce chunk through DRAM to transpose. uhd rows are
                    # (gg, cc, b) so both sides are linear:
                    #   write dst addr = gg*32768 + p*256      (p = cc*16+b)
                    #   read  src addr = p'*4096 + b*256       (p' = gg*8+cc)
                    nc.sync.dma_start(
                        sub(uhd[:], sg * CHW,
                            [[256, 128], [128 * 256, NGRP], [1, 256]]),
                        sub(stg[:], 0, [[pstS, 128], [256, NGRP], [1, 256]]))
                    nc.sync.dma_start(
                        sub(Aap, sg * FREE, [[pstA, 128], [1, FREE]]),
                        sub(uhd[:], sg * CHW, [[FREE, 128], [1, FREE]]))
                # copy iter-0 s_j out of PSUM before the pools close
                nc.vector.tensor_copy(s0[:], psj0[:])

            # ---------------- routing ----------------
            with tc.tile_pool(name="rt", bufs=1) as rt, \
                 tc.tile_pool(name="tb", bufs=2) as tb, \
                 tc.tile_pool(name="prp", bufs=2) as prp, \
                 tc.tile_pool(name="pss", bufs=1, space="PSUM") as pss:
                cij = rt.tile([128, NCHUNK, NUM_UNIT], f32)
                cijb = rt.tile([128, NCHUNK, NUM_UNIT], bf16)
                smax = rt.tile([128, NCHUNK], f32)
                ssum = rt.tile([128, NCHUNK], f32)
                ar_sb = rt.tile([128, NCHUNK, NUM_UNIT], f32)
                prodD = rt.tile([128, FREE], bf16)   # DVE agreement scratch
                pstPD = prodD[:].ap[0][0]

                def squash(src_ap, pst_src, final, rnd):
                    """squash over s of src [16p, (x, s)] (x=b or n outer);
                    returns v bf16 tile [16, 256] (or writes vout if final)."""
                    s2 = tb.tile([16, NS], f32, tag="s2", name=f"s2_{rnd}")
                    nc.vector.tensor_tensor(s2[:], src_ap, src_ap,
                                            op=ALU.mult)
                    sq = tb.tile([16, 16], f32, tag="sq", name=f"sq_{rnd}")
                    nc.vector.tensor_reduce(
                        sq[:], sub(s2[:], 0, [[s2[:].ap[0][0], 16], [16, 16],
                                              [1, 16]]),
                        axis=AX.X, op=ALU.add)
                    rsq = tb.tile([16, 16], f32, tag="rsq", name=f"rsq_{rnd}")
                    nc.scalar.sqrt(rsq[:], sq[:])
                    den = tb.tile([16, 16], f32, tag="den", name=f"den_{rnd}")
                    nc.vector.scalar_tensor_tensor(den[:], sq[:], 1.0, rsq[:],
                                                   op0=ALU.add, op1=ALU.mult)
                    nc.vector.reciprocal(den[:], den[:])
                    fac = tb.tile([16, 16], f32, tag="fac", name=f"fac_{rnd}")
                    nc.vector.tensor_tensor(fac[:], sq[:], den[:], op=ALU.mult)
                    pstF = fac[:].ap[0][0]
                    fb = sub(fac[:], 0, [[pstF, 16], [1, 16], [0, 16]])
                    if final:
                        v32 = tb.tile([16, NS], f32, tag="v32", name="v32")
                        nc.vector.tensor_tensor(v32[:], src_ap, fb,
                                                op=ALU.mult)
                        nc.sync.dma_start(vout_t[:], v32[:])
                        return None
                    v16 = tb.tile([16, NS], bf16, tag="v16", name=f"v16_{rnd}")
                    nc.vector.tensor_tensor(v16[:], src_ap, fb, op=ALU.mult)
                    return v16

                def broadcast_v(v16, rnd, order_nbs):
                    """v16 [16, 256] -> vb [128, (b,n,s)] via DRAM bounce.
                    order_nbs: True if v16 is [n, (b,s)], else [b, (n,s)]."""
                    vfl = drampool.tile([FREE], bf16, name=f"vfl_{rnd}",
                                        tag=f"vfl{rnd}")
                    pstV = v16[:].ap[0][0]
                    if order_nbs:
                        # iterate (n, b, s): dst offset b*256 + n*16 + s
                        nc.sync.dma_start(
                            sub(vfl[:], 0, [[16, 16], [256, BL], [1, 16]]),
                            sub(v16[:], 0, [[pstV, 16], [16, BL], [1, 16]]))
                    else:
                        nc.sync.dma_start(
                            sub(vfl[:], 0, [[256, 16], [1, 256]]),
                            sub(v16[:], 0, [[pstV, 16], [1, 256]]))
                    nc.sync.dma_start(
                        sub(vb[:], 0, [[pstVB, 128], [1, FREE]]),
                        sub(vfl[:], 0, [[0, 128], [1, FREE]]))

                def junk_mm(psj_t, src_ap, tag):
                    """tiny keep-warm matmul writing psj[0:1, 0:16]."""
                    nc.tensor.matmul(psj_t[0:1, 0:16], sd[:, 0:1], src_ap,
                                     start=True, stop=True)

                # iter-0 squash from s0 [b, (n,s)]
                v16 = squash(sub(s0[:], 0, [[s0[:].ap[0][0], 16], [1, 256]]),
                             s0[:].ap[0][0], final=(niters == 1), rnd=0)
                if niters > 1:
                    broadcast_v(v16, 0, order_nbs=False)

                HALF1 = (0, 1, 2, 3, 4)
                HALF2 = (5, 6, 7, 8)

                for rnd in range(1, niters):
                    psj = pss.tile([NUM_UNIT, FREE], f32, tag="psj",
                                   name=f"psj_{rnd}")
                    pstP = psj[:].ap[0][0]

                    # ---- agreement: uv[c, n] = sum_{b,s} A * vb ----
                    dve_chunks = [k for k in range(NCHUNK)
                                  if k not in POOL_CHUNKS]
                    prodP = {}
                    for k in POOL_CHUNKS:
                        t = prp.tile([128, FREE], bf16, tag="prodP",
                                     name=f"prodP_{rnd}_{k}")
                        nc.gpsimd.tensor_tensor(
                            t[:], sub(Aap, k * FREE, [[pstA, 128], [1, FREE]]),
                            vb[:], op=ALU.mult)
                        prodP[k] = t

                    def folds(pt, pst, k):
                        # fold b: 2048, 1024, 512, 256 then s: 128,64,32,16
                        for sz in (2048, 1024, 512, 256):
                            nc.vector.tensor_tensor(
                                sub(pt, 0, [[pst, 128], [1, sz]]),
                                sub(pt, 0, [[pst, 128], [1, sz]]),
                                sub(pt, sz, [[pst, 128], [1, sz]]),
                                op=ALU.add)
                        for sz in (8, 4, 2):
                            nc.vector.tensor_tensor(
                                sub(pt, 0, [[pst, 128], [16, 16], [1, sz]]),
                                sub(pt, 0, [[pst, 128], [16, 16], [1, sz]]),
                                sub(pt, sz, [[pst, 128], [16, 16], [1, sz]]),
                                op=ALU.add)
                        nc.vector.tensor_tensor(
                            uv[:, k, :],
                            sub(pt, 0, [[pst, 128], [16, 16], [1, 1]]),
                            sub(pt, 1, [[pst, 128], [16, 16], [1, 1]]),
                            op=ALU.add)

                    # DVE order: interleave pool-chunk folds early so their
                    # prodP buffers (bufs=2) free up for later pool mults.
                    pool_after = {1: 6, 3: 7, 5: 8}   # after dve chunk k
                    for k in dve_chunks:
                        nc.vector.tensor_tensor(
                            prodD[:],
                            sub(Aap, k * FREE, [[pstA, 128], [1, FREE]]),
                            vb[:], op=ALU.mult)
                        junk_mm(psj, prodD[:, 0:16], f"jm_{rnd}_{k}")
                        folds(prodD[:], pstPD, k)
                        junk_mm(psj, prodD[:, 16:32], f"jf_{rnd}_{k}")
                        if k in pool_after:
                            kp = pool_after[k]
                            pt = prodP[kp][:]
                            folds(pt, pt.ap[0][0], kp)
                            junk_mm(psj, pt[:, 0:16], f"jp_{rnd}_{kp}")

                    # ---- allreduce (split halves), b_ij update, softmax ----
                    for hi, half in enumerate((HALF1, HALF2)):
                        k0, nk = half[0], len(half)
                        w = nk * NUM_UNIT
                        arbi = drampool.tile([128, w], f32,
                                             name=f"arbi_{rnd}_{hi}",
                                             tag=f"arbi{rnd}{hi}")
                        arbo = drampool.tile([128, w], f32,
                                             addr_space="Shared",
                                             name=f"arbo_{rnd}_{hi}",
                                             tag=f"arbo{rnd}{hi}")
                        nc.gpsimd.dma_start(
                            arbi[:], sub(uv[:], k0 * NUM_UNIT,
                                         [[uv[:].ap[0][0], 128], [1, w]]))
                        if single_core:
                            nc.gpsimd.dma_start(arbo[:], arbi[:])
                        else:
                            nc.gpsimd.collective_compute(
                                "AllReduce", ALU.add,
                                replica_groups=[list(range(NCORES))],
                                ins=[arbi.opt()], outs=[arbo.opt()])
                        nc.sync.dma_start(
                            sub(ar_sb[:], k0 * NUM_UNIT,
                                [[ar_sb[:].ap[0][0], 128], [1, w]]),
                            arbo[:])
                        # b_ij += AR/B
                        bsl = sub(b_ij[:], k0 * NUM_UNIT,
                                  [[b_ij[:].ap[0][0], 128], [1, w]])
                        nc.vector.scalar_tensor_tensor(
                            bsl, sub(ar_sb[:], k0 * NUM_UNIT,
                                     [[ar_sb[:].ap[0][0], 128], [1, w]]),
                            1.0 / B, bsl, op0=ALU.mult, op1=ALU.add)
                        # softmax over n for this half's chunks
                        csl = lambda t, w_=w: sub(
                            t[:], k0 * NUM_UNIT, [[t[:].ap[0][0], 128],
                                                  [16, nk], [1, 16]])
                        ssl = lambda t: sub(t[:], k0,
                                            [[t[:].ap[0][0], 128], [1, nk]])
                        nc.vector.tensor_reduce(ssl(smax), csl(b_ij),
                                                axis=AX.X, op=ALU.max)
                        nc.vector.tensor_tensor(
                            csl(cij), csl(b_ij),
                            sub(smax[:], k0, [[smax[:].ap[0][0], 128],
                                              [1, nk], [0, 16]]),
                            op=ALU.subtract)
                        nc.scalar.activation(csl(cij), csl(cij), ACT.Exp)
                        nc.vector.tensor_reduce(ssl(ssum), csl(cij),
                                                axis=AX.X, op=ALU.add)
                        nc.vector.reciprocal(ssl(ssum), ssl(ssum))
                        nc.vector.tensor_tensor(
                            csl(cij), csl(cij),
                            sub(ssum[:], k0, [[ssum[:].ap[0][0], 128],
                                              [1, nk], [0, 16]]),
                            op=ALU.mult)
                        nc.vector.tensor_copy(csl(cijb), csl(cij))
                        # s_j for this half's chunks
                        for k in half:
                            for j in range(FREE // 512):
                                nc.tensor.matmul(
                                    psj[:, j * 512:(j + 1) * 512],
                                    cijb[:, k, :],
                                    sub(Aap, k * FREE + j * 512,
                                        [[pstA, 128], [1, 512]]),
                                    start=(k == 0), stop=(k == NCHUNK - 1))

                    # diag extract: s_t[n, b, s] = psj[n, (b, n, s)].
                    # Engine APs can't take the diag partition step, so copy
                    # PSUM->SBUF (DVE+ACT halves) then one diag DMA.
                    sjf = tb.tile([NUM_UNIT, FREE], f32, tag="sjf", bufs=1,
                                  name=f"sjf_{rnd}")
                    pstJ = sjf[:].ap[0][0]
                    nc.vector.tensor_copy(sjf[:, :FREE // 2],
                                          psj[:, :FREE // 2])
                    nc.scalar.copy(sjf[:, FREE // 2:], psj[:, FREE // 2:])
                    s_t = tb.tile([NUM_UNIT, BL, UNIT_SIZE], f32, tag="s_t",
                                  name=f"s_t{rnd}")
                    nc.sync.dma_start(
                        s_t[:], sub(sjf[:], 0, [[pstJ + 16, 16], [256, BL],
                                                [1, 16]]))
                    pstST = s_t[:].ap[0][0]
                    v16 = squash(sub(s_t[:], 0, [[pstST, 16], [1, 256]]),
                                 pstST, final=(rnd == niters - 1), rnd=rnd)
                    if rnd < niters - 1:
                        broadcast_v(v16, rnd, order_nbs=True)

    nc.compile()
    return nc


def _prep(x, weight):
    import ml_dtypes
    bf = ml_dtypes.bfloat16
    wr = np.ascontiguousarray(
        weight.reshape(NCHUNK * NGRP, 8, NUM_UNIT, UNIT_SIZE, IN_UNIT)
        .transpose(0, 1, 4, 2, 3).reshape(NCHUNK * NGRP * 128, 256)
    ).astype(bf)
    sd = (np.tile(np.eye(BL, dtype=np.float32), (8, 1)) / NUM_UNIT).astype(bf)
    in_maps = []
    for c in range(NCORES):
        xs = x[c * BL:(c + 1) * BL]          # [BL, i, C]
        # xv[sg, cc, i, gg, b] = x[b, i, sg*128 + gg*8 + cc]
        xv = xs.reshape(BL, IN_UNIT, NCHUNK, NGRP, 8).transpose(2, 4, 1, 3, 0)
        # dense block-diag: xbd[sg, (cc,i), gg, (cc2,b)], nonzero iff cc2==cc
        xbd = np.zeros((NCHUNK, 8, IN_UNIT, NGRP, 8, BL), np.float32)
        for cc in range(8):
            xbd[:, cc, :, :, cc, :] = xv[:, cc]
        xbd = np.ascontiguousarray(
            xbd.reshape(NCHUNK, 128, NGRP * 128)).astype(bf)
        in_maps.append({"wr": wr, "xbd": xbd, "sd": sd})
    return in_maps


def kernel(x, x_original, weight, mode, epoch, _trace=False):
    from concourse.bass_utils import run_bass_kernel_spmd

    x = np.asarray(x, dtype=np.float32)
    weight = np.asarray(weight, dtype=np.float32)
    if "nc" not in _cache:
        _cache["nc"] = _build()
    nc = _cache["nc"]
    in_maps = _prep(x, weight)
    res = run_bass_kernel_spmd(nc, in_maps, core_ids=list(range(NCORES)),
                               trace=_trace)
    _cache["last_result"] = res
    out = np.empty((B, NUM_UNIT, UNIT_SIZE), np.float32)
    for c in range(NCORES):
        vo = res.results[c]["vout"].reshape(NUM_UNIT, BL, UNIT_SIZE)
        out[c * BL:(c + 1) * BL] = vo.transpose(1, 0, 2)
    return out[..., None]


# revision 16
# speedup vs baseline: 1.8983x; 1.1195x over previous
"""CapsuleLayer dynamic-routing kernel for 8 Trainium2 NeuronCores.

Sharding: data-parallel over batch (16 batches/core), weight replicated.
  u_hat[b,c,n,s] = sum_i W[c,n,s,i] * x[b,i,c]   (PE, bf16, block-diag x)
  3 routing iterations; b_ij update takes a mean over the full batch via
  AllReduce (split in channel-halves so it overlaps compute).

v2 design vs baseline:
  - everything bf16 (tol 2e-2; measured rel err ~1e-3): halves DMA + 2x DVE
  - u_hat transposed to [c-part, (b,n,s)] via one SBUF->SBUF DMA per chunk
    (512B runs) instead of a DRAM round-trip
  - iteration-0 s_j computed during production from the staging tiles with a
    delta/16 stationary (PE idle time), output directly [b,(n,s)] (no diag)
  - agreement = DVE tensor_tensor mult + pairwise fold tree (bf16 2x mode);
    3 chunk-mults offloaded to GPSIMD
  - diag extract for s_j via a single strided DVE copy from PSUM
  - v broadcast via DRAM bounce + stride-0 broadcast DMA
  - tiny keep-warm matmuls gated on DVE steps hold the PE clock at max
"""

import sys

sys.path.insert(0, "/opt/trn_rl_repo")

import numpy as np

B, IN_UNIT, IN_CHANNEL = 128, 16, 1152
NUM_UNIT, UNIT_SIZE = 16, 16
NCORES = 8
BL = B // NCORES               # 16 batches per core
NCHUNK = IN_CHANNEL // 128     # 9 c-chunks
NGRP = 16                      # groups of 8 channels per chunk
NS = NUM_UNIT * UNIT_SIZE      # 256
FREE = BL * NS                 # 4096 = (b, n, s) free size per chunk
POOL_CHUNKS = (6, 7, 8)        # agreement mults done on gpsimd

_cache = {}


def _build(single_core=False, niters=3):
    import concourse.bass as bass
    import concourse.bacc as bacc
    import concourse.mybir as mybir
    import concourse.tile as tile

    f32 = mybir.dt.float32
    bf16 = mybir.dt.bfloat16
    ALU = mybir.AluOpType
    AX = mybir.AxisListType
    ACT = mybir.ActivationFunctionType

    def sub(ap, off, dims, cast=None):
        a = bass.AP(ap.tensor, ap.offset + off, [list(d) for d in dims])
        return a.bitcast(cast) if cast is not None else a

    nc = bacc.Bacc("TRN2", target_bir_lowering=False, debug=False,
                   num_devices=1 if single_core else NCORES)

    wr_t = nc.dram_tensor("wr", [NCHUNK * NGRP * 128, 256], bf16,
                          kind="ExternalInput")
    xbd_t = nc.dram_tensor("xbd", [NCHUNK, 128, NGRP * 128], bf16,
                           kind="ExternalInput")
    sd_t = nc.dram_tensor("sd", [128, BL], bf16, kind="ExternalInput")
    vout_t = nc.dram_tensor("vout", [NUM_UNIT, BL * UNIT_SIZE], f32,
                            kind="ExternalOutput")

    with tile.TileContext(nc) as tc:
        with tc.tile_pool(name="persist", bufs=1) as persist, \
             tc.tile_pool(name="drampool", bufs=1, space="DRAM") as drampool:
            A = persist.tile([128, NCHUNK, BL, NUM_UNIT, UNIT_SIZE], bf16)
            Aap = A[:]
            pstA = Aap.ap[0][0]
            b_ij = persist.tile([128, NCHUNK, NUM_UNIT], f32)
            uv = persist.tile([128, NCHUNK, NUM_UNIT], f32)
            vb = persist.tile([128, BL, NUM_UNIT, UNIT_SIZE], bf16)
            pstVB = vb[:].ap[0][0]
            sd = persist.tile([128, BL], bf16)
            s0 = persist.tile([BL, NS], f32)     # iter-0 s_j, SBUF copy
            nc.gpsimd.memset(b_ij[:], 0.0)
            nc.sync.dma_start(sd[:], sd_t[:])

            # ---------------- production + iter-0 s_j ----------------
            CHW = NGRP * 128 * 256          # uhd elements per chunk
            uhd = drampool.tile([NCHUNK, NGRP, 8, BL, 256], bf16,
                                name="uhd")  # rows (gg, cc, b)
            with tc.tile_pool(name="bdp", bufs=2) as bdp, \
                 tc.tile_pool(name="wp", bufs=2) as wp, \
                 tc.tile_pool(name="stgp", bufs=2) as stgp, \
                 tc.tile_pool(name="psp", bufs=3, space="PSUM") as psp, \
                 tc.tile_pool(name="ps0", bufs=1, space="PSUM") as ps0:
                psj0 = ps0.tile([BL, NS], f32)   # s_j iter0: [b, (n,s)]
                for sg in range(NCHUNK):
                    # dense host-built block-diag xT: bd[(cc,i), (gg, cc*16+b)]
                    bd = bdp.tile([128, NGRP, 128], bf16, tag="bd",
                                  name=f"bd_{sg}")
                    pstB = bd[:].ap[0][0]
                    nc.sync.dma_start(
                        sub(bd[:], 0, [[pstB, 128], [1, NGRP * 128]]),
                        sub(xbd_t[:], sg * 128 * NGRP * 128,
                            [[NGRP * 128, 128], [1, NGRP * 128]]))
                    # weights for the chunk: wt[(cc,i), gq, (n,s)]
                    wt = wp.tile([128, NGRP, 256], bf16, tag="wt",
                                 name=f"wt_{sg}")
                    nc.sync.dma_start(
                        wt[:], sub(wr_t[:], sg * NGRP * 128 * 256,
                                   [[256, 128], [128 * 256, NGRP], [1, 256]]))
                    stg = stgp.tile([128, NGRP, 256], bf16, tag="stg",
                                    name=f"stg_{sg}")
                    pstS = stg[:].ap[0][0]
                    for q in range(4):      # 4 psum tiles of 4 groups
                        ps = psp.tile([128, 4 * 256], f32, tag="pp",
                                      name=f"pp_{sg}_{q}")
                        for g4 in range(4):
                            gg = q * 4 + g4
                            nc.tensor.matmul(ps[:, g4 * 256:(g4 + 1) * 256],
                                             bd[:, gg, :], wt[:, gg, :],
                                             start=True, stop=True)
                        eng = nc.vector if q % 2 == 0 else nc.scalar
                        if q % 2 == 0:
                            eng.tensor_copy(
                                sub(stg[:], q * 4 * 256,
                                    [[pstS, 128], [1, 1024]]),
                                ps[:])
                        else:
                            eng.copy(
                                sub(stg[:], q * 4 * 256,
                                    [[pstS, 128], [1, 1024]]),
                                ps[:])
                    # iter-0 s_j partials: psj0[b,(n,s)] += sd^T @ stg
                    for gg in range(NGRP):
                        nc.tensor.matmul(psj0[:], sd[:], stg[:, gg, :],
                                         start=(sg == 0 and gg == 0),
                                         stop=(sg == NCHUNK - 1
                                               and gg == NGRP - 1))
                    # bounce chunk through DRAM to transpose. uhd rows are
                    # (gg, cc, b) so both sides are linear:
                    #   write dst addr = gg*32768 + p*256      (p = cc*16+b)
                    #   read  src addr = p'*4096 + b*256       (p' = gg*8+cc)
                    nc.sync.dma_start(
                        sub(uhd[:], sg * CHW,
                            [[256, 128], [128 * 256, NGRP], [1, 256]]),
                        sub(stg[:], 0, [[pstS, 128], [256, NGRP], [1, 256]]))
                    nc.sync.dma_start(
                        sub(Aap, sg * FREE, [[pstA, 128], [1, FREE]]),
                        sub(uhd[:], sg * CHW, [[FREE, 128], [1, FREE]]))
                # copy iter-0 s_j out of PSUM before the pools close
                nc.vector.tensor_copy(s0[:], psj0[:])

            # ---------------- routing ----------------
            with tc.tile_pool(name="rt", bufs=1) as rt, \
                 tc.tile_pool(name="tb", bufs=2) as tb, \
                 tc.tile_pool(name="pss", bufs=1, space="PSUM") as pss:
                cij = rt.tile([128, NCHUNK, NUM_UNIT], f32)
                cijb = rt.tile([128, NCHUNK, NUM_UNIT], bf16)
                smax = rt.tile([128, NCHUNK], f32)
                ssum = rt.tile([128, NCHUNK], f32)
                ar_sb = rt.tile([128, NCHUNK, NUM_UNIT], f32)
                prodD = rt.tile([128, FREE], bf16)   # DVE agreement scratch
                prodP = rt.tile([128, FREE], bf16)   # Pool agreement scratch
                jch = rt.tile([128, 16], bf16)       # keep-warm gate mirror
                pstPD = prodD[:].ap[0][0]
                pstPP = prodP[:].ap[0][0]
                nc.gpsimd.memset(jch[:].bitcast(f32), 0.0)

                jp = pss.tile([1, 16], f32)          # junk-matmul PSUM target

                def junk(src_ap):
                    """tiny keep-warm matmul; holds the PE clock streak."""
                    nc.tensor.matmul(jp[0:1, 0:16], sd[:, 0:1], src_ap,
                                     start=True, stop=True)

                def jgate(src_f32_ap, tag):
                    """mirror a just-computed f32 value into bf16 + junk."""
                    p = src_f32_ap.ap[0][1]
                    nc.vector.tensor_copy(
                        sub(jch[:], 0, [[jch[:].ap[0][0], p], [1, 16]]),
                        src_f32_ap)
                    junk(jch[:])

                def agr_chunk_dve(k):
                    A_k = sub(Aap, k * FREE, [[pstA, 128], [1, FREE]])
                    nc.vector.tensor_tensor(prodD[:], A_k, vb[:], op=ALU.mult)
                    junk(prodD[:, 2048:2064])
                    for sz in (2048, 1024, 512, 256):
                        nc.vector.tensor_tensor(
                            sub(prodD[:], 0, [[pstPD, 128], [1, sz]]),
                            sub(prodD[:], 0, [[pstPD, 128], [1, sz]]),
                            sub(prodD[:], sz, [[pstPD, 128], [1, sz]]),
                            op=ALU.add)
                        if sz == 2048:
                            junk(prodD[:, 1024:1040])
                        if sz == 512:
                            junk(prodD[:, 256:272])
                    nc.vector.tensor_reduce(
                        uv[:, k, :],
                        sub(prodD[:], 0, [[pstPD, 128], [16, 16], [1, 16]]),
                        axis=AX.X, op=ALU.add)
                    junk(prodD[:, 2064:2080])

                def agr_chunk_pool(k):
                    A_k = sub(Aap, k * FREE, [[pstA, 128], [1, FREE]])
                    nc.gpsimd.tensor_tensor(prodP[:], A_k, vb[:], op=ALU.mult)
                    for sz in (2048, 1024, 512, 256):
                        nc.gpsimd.tensor_tensor(
                            sub(prodP[:], 0, [[pstPP, 128], [1, sz]]),
                            sub(prodP[:], 0, [[pstPP, 128], [1, sz]]),
                            sub(prodP[:], sz, [[pstPP, 128], [1, sz]]),
                            op=ALU.add)
                    for sz in (8, 4, 2):
                        nc.gpsimd.tensor_tensor(
                            sub(prodP[:], 0, [[pstPP, 128], [16, 16], [1, sz]]),
                            sub(prodP[:], 0, [[pstPP, 128], [16, 16], [1, sz]]),
                            sub(prodP[:], sz, [[pstPP, 128], [16, 16],
                                               [1, sz]]),
                            op=ALU.add)
                    nc.gpsimd.tensor_tensor(
                        uv[:, k, :],
                        sub(prodP[:], 0, [[pstPP, 128], [16, 16], [1, 1]]),
                        sub(prodP[:], 1, [[pstPP, 128], [16, 16], [1, 1]]),
                        op=ALU.add)

                def ar_half(rnd, hi, half):
                    """AllReduce of uv chunks `half` (DMA + collective only)."""
                    k0, nk = half[0], len(half)
                    w = nk * NUM_UNIT
                    arbi = drampool.tile([128, w], f32, name=f"arbi_{rnd}_{hi}",
                                         tag=f"arbi{rnd}{hi}")
                    arbo = drampool.tile([128, w], f32, addr_space="Shared",
                                         name=f"arbo_{rnd}_{hi}",
                                         tag=f"arbo{rnd}{hi}")
                    nc.gpsimd.dma_start(
                        arbi[:], sub(uv[:], k0 * NUM_UNIT,
                                     [[uv[:].ap[0][0], 128], [1, w]]))
                    if single_core:
                        nc.gpsimd.dma_start(arbo[:], arbi[:])
                    else:
                        nc.gpsimd.collective_compute(
                            "AllReduce", ALU.add,
                            replica_groups=[list(range(NCORES))],
                            ins=[arbi.opt()], outs=[arbo.opt()])
                    nc.sync.dma_start(
                        sub(ar_sb[:], k0 * NUM_UNIT,
                            [[ar_sb[:].ap[0][0], 128], [1, w]]),
                        arbo[:])

                def softmax_half(rnd, hi, half):
                    k0, nk = half[0], len(half)
                    w = nk * NUM_UNIT
                    jgate(sub(ar_sb[:], k0 * NUM_UNIT,
                              [[ar_sb[:].ap[0][0], 128], [1, 16]]),
                          f"jar{rnd}{hi}")
                    bsl = sub(b_ij[:], k0 * NUM_UNIT,
                              [[b_ij[:].ap[0][0], 128], [1, w]])
                    nc.vector.scalar_tensor_tensor(
                        bsl, sub(ar_sb[:], k0 * NUM_UNIT,
                                 [[ar_sb[:].ap[0][0], 128], [1, w]]),
                        1.0 / B, bsl, op0=ALU.mult, op1=ALU.add)
                    csl = lambda t: sub(t[:], k0 * NUM_UNIT,
                                        [[t[:].ap[0][0], 128], [16, nk],
                                         [1, 16]])
                    ssl = lambda t: sub(t[:], k0,
                                        [[t[:].ap[0][0], 128], [1, nk]])
                    nc.vector.tensor_reduce(ssl(smax), csl(b_ij), axis=AX.X,
                                            op=ALU.max)
                    nc.vector.tensor_tensor(
                        csl(cij), csl(b_ij),
                        sub(smax[:], k0, [[smax[:].ap[0][0], 128], [1, nk],
                                          [0, 16]]),
                        op=ALU.subtract)
                    nc.scalar.activation(csl(cij), csl(cij), ACT.Exp)
                    nc.vector.tensor_reduce(ssl(ssum), csl(cij), axis=AX.X,
                                            op=ALU.add)
                    nc.vector.reciprocal(ssl(ssum), ssl(ssum))
                    nc.vector.tensor_tensor(
                        csl(cij), csl(cij),
                        sub(ssum[:], k0, [[ssum[:].ap[0][0], 128], [1, nk],
                                          [0, 16]]),
                        op=ALU.mult)
                    nc.vector.tensor_copy(csl(cijb), csl(cij))
                    junk(cijb[:, k0, :])

                def sj_mms(psjH_t, hb, ks):
                    for k in ks:
                        for j in range(4):
                            nc.tensor.matmul(
                                psjH_t[:, j * 512:(j + 1) * 512],
                                cijb[:, k, :],
                                sub(Aap, k * FREE + hb * 2048 + j * 512,
                                    [[pstA, 128], [1, 512]]),
                                start=(k == 0), stop=(k == NCHUNK - 1))

                def squash(src_ap, pst_src, nb, final, rnd, hb, order_nbs):
                    """squash over s of src [16, (x, s)], x of size nb.
                    order_nbs: src rows are n (True) or b (False)."""
                    W = nb * 16
                    s2 = tb.tile([16, W], f32, tag="s2", name=f"s2_{rnd}_{hb}")
                    nc.vector.tensor_tensor(s2[:], src_ap, src_ap,
                                            op=ALU.mult)
                    sq = tb.tile([16, nb], f32, tag="sq",
                                 name=f"sq_{rnd}_{hb}")
                    nc.vector.tensor_reduce(
                        sq[:], sub(s2[:], 0, [[s2[:].ap[0][0], 16], [16, nb],
                                              [1, 16]]),
                        axis=AX.X, op=ALU.add)
                    jgate(sub(sq[:], 0, [[sq[:].ap[0][0], 16], [1, nb],
                                         [0, 16 // nb]]),
                          f"jsq{rnd}{hb}")
                    rsq = tb.tile([16, nb], f32, tag="rsq",
                                  name=f"rsq_{rnd}_{hb}")
                    nc.scalar.sqrt(rsq[:], sq[:])
                    den = tb.tile([16, nb], f32, tag="den",
                                  name=f"den_{rnd}_{hb}")
                    nc.vector.scalar_tensor_tensor(den[:], sq[:], 1.0, rsq[:],
                                                   op0=ALU.add, op1=ALU.mult)
                    nc.vector.reciprocal(den[:], den[:])
                    fac = tb.tile([16, nb], f32, tag="fac",
                                  name=f"fac_{rnd}_{hb}")
                    nc.vector.tensor_tensor(fac[:], sq[:], den[:], op=ALU.mult)
                    pstF = fac[:].ap[0][0]
                    fb = sub(fac[:], 0, [[pstF, 16], [1, nb], [0, 16]])
                    if final:
                        v32 = tb.tile([16, W], f32, tag="v32",
                                      name=f"v32_{hb}")
                        nc.vector.tensor_tensor(v32[:], src_ap, fb,
                                                op=ALU.mult)
                        nc.sync.dma_start(
                            sub(vout_t[:], hb * 128,
                                [[256, 16], [1, W]]),
                            v32[:])
                        return
                    v16 = tb.tile([16, W], bf16, tag="v16",
                                  name=f"v16_{rnd}_{hb}")
                    nc.vector.tensor_tensor(v16[:], src_ap, fb, op=ALU.mult)
                    jgate(sub(fac[:], 0, [[pstF, 16], [1, nb], [0, 16 // nb]]),
                          f"jv{rnd}{hb}")
                    # flatten to DRAM in (b, n, s) order, then broadcast
                    vfl = drampool.tile([FREE], bf16, name=f"vfl_{rnd}_{hb}",
                                        tag=f"vfl{rnd}{hb}")
                    pstV = v16[:].ap[0][0]
                    if order_nbs:    # v16 is [n, (b_half, s)]
                        nc.sync.dma_start(
                            sub(vfl[:], hb * 8 * 256,
                                [[16, 16], [256, nb], [1, 16]]),
                            sub(v16[:], 0, [[pstV, 16], [16, nb], [1, 16]]))
                    else:            # v16 is [b, (n, s)] (iter 0, full)
                        nc.sync.dma_start(
                            sub(vfl[:], 0, [[256, 16], [1, 256]]),
                            sub(v16[:], 0, [[pstV, 16], [1, 256]]))
                    nc.sync.dma_start(
                        sub(vb[:], hb * 8 * 256 if order_nbs else 0,
                            [[pstVB, 128], [1, W * 16 if order_nbs else FREE]]),
                        sub(vfl[:], hb * 8 * 256 if order_nbs else 0,
                            [[0, 128], [1, W * 16 if order_nbs else FREE]]))

                def diag_squash_half(rnd, hb, psjH_t, final):
                    sjf = tb.tile([NUM_UNIT, FREE // 2], f32, tag="sjf",
                                  bufs=1, name=f"sjf_{rnd}_{hb}")
                    pstJ = sjf[:].ap[0][0]
                    nc.vector.tensor_copy(sjf[:, :1024], psjH_t[:, :1024])
                    nc.scalar.copy(sjf[:, 1024:], psjH_t[:, 1024:])
                    jgate(sub(sjf[:], 0, [[pstJ, 16], [1, 16]]),
                          f"jsjf{rnd}{hb}")
                    s_t = tb.tile([NUM_UNIT, 8, UNIT_SIZE], f32, tag="s_t",
                                  name=f"s_t{rnd}_{hb}")
                    nc.sync.dma_start(
                        s_t[:], sub(sjf[:], 0, [[pstJ + 16, 16], [256, 8],
                                                [1, 16]]))
                    squash(sub(s_t[:], 0, [[s_t[:].ap[0][0], 16], [1, 128]]),
                           s_t[:].ap[0][0], nb=8, final=final, rnd=rnd, hb=hb,
                           order_nbs=True)

                # iter-0 squash from s0 [b, (n,s)]
                squash(sub(s0[:], 0, [[s0[:].ap[0][0], 16], [1, 256]]),
                       s0[:].ap[0][0], nb=16, final=(niters == 1), rnd=0,
                       hb=0, order_nbs=False)

                HALF1 = (0, 1, 2, 3, 4)
                HALF2 = (5, 6, 7, 8)
                POOLK = (7, 8)
                DVE1 = tuple(k for k in HALF1 if k not in POOLK)
                DVE2 = tuple(k for k in HALF2 if k not in POOLK)

                for rnd in range(1, niters):
                    final = rnd == niters - 1
                    agr_chunk_pool(POOLK[0])
                    agr_chunk_pool(POOLK[1])
                    for k in DVE1:
                        agr_chunk_dve(k)
                    ar_half(rnd, 0, HALF1)
                    for k in DVE2:
                        agr_chunk_dve(k)
                    ar_half(rnd, 1, HALF2)
                    psjH1 = pss.tile([NUM_UNIT, FREE // 2], f32, tag="psjH",
                                     name=f"psjH_{rnd}_0")
                    softmax_half(rnd, 0, HALF1)
                    sj_mms(psjH1, 0, HALF1)
                    softmax_half(rnd, 1, HALF2)
                    sj_mms(psjH1, 0, HALF2)
                    diag_squash_half(rnd, 0, psjH1, final)
                    psjH2 = pss.tile([NUM_UNIT, FREE // 2], f32, tag="psjH",
                                     name=f"psjH_{rnd}_1")
                    sj_mms(psjH2, 1, HALF1 + HALF2)
                    diag_squash_half(rnd, 1, psjH2, final)

    nc.compile()
    return nc


def _prep(x, weight):
    import ml_dtypes
    bf = ml_dtypes.bfloat16
    wr = np.ascontiguousarray(
        weight.reshape(NCHUNK * NGRP, 8, NUM_UNIT, UNIT_SIZE, IN_UNIT)
        .transpose(0, 1, 4, 2, 3).reshape(NCHUNK * NGRP * 128, 256)
    ).astype(bf)
    sd = (np.tile(np.eye(BL, dtype=np.float32), (8, 1)) / NUM_UNIT).astype(bf)
    in_maps = []
    for c in range(NCORES):
        xs = x[c * BL:(c + 1) * BL]          # [BL, i, C]
        # xv[sg, cc, i, gg, b] = x[b, i, sg*128 + gg*8 + cc]
        xv = xs.reshape(BL, IN_UNIT, NCHUNK, NGRP, 8).transpose(2, 4, 1, 3, 0)
        # dense block-diag: xbd[sg, (cc,i), gg, (cc2,b)], nonzero iff cc2==cc
        xbd = np.zeros((NCHUNK, 8, IN_UNIT, NGRP, 8, BL), np.float32)
        for cc in range(8):
            xbd[:, cc, :, :, cc, :] = xv[:, cc]
        xbd = np.ascontiguousarray(
            xbd.reshape(NCHUNK, 128, NGRP * 128)).astype(bf)
        in_maps.append({"wr": wr, "xbd": xbd, "sd": sd})
    return in_maps


def kernel(x, x_original, weight, mode, epoch, _trace=False):
    from concourse.bass_utils import run_bass_kernel_spmd

    x = np.asarray(x, dtype=np.float32)
    weight = np.asarray(weight, dtype=np.float32)
    if "nc" not in _cache:
        _cache["nc"] = _build()
    nc = _cache["nc"]
    in_maps = _prep(x, weight)
    res = run_bass_kernel_spmd(nc, in_maps, core_ids=list(range(NCORES)),
                               trace=_trace)
    _cache["last_result"] = res
    out = np.empty((B, NUM_UNIT, UNIT_SIZE), np.float32)
    for c in range(NCORES):
        vo = res.results[c]["vout"].reshape(NUM_UNIT, BL, UNIT_SIZE)
        out[c * BL:(c + 1) * BL] = vo.transpose(1, 0, 2)
    return out[..., None]
